# revision 25
# baseline (speedup 1.0000x reference)
"""Trainium2 Bass kernel for a seq2seq LSTM (1-step encoder + T-step decoder + FC).

Model (B=512, I=256, H=1024, O=128, T=100):
  h,c   = LSTMCell(x, 0, 0; enc_Wih, enc_Whh, enc_b)          # encoder
  loop t in 0..T-1:  h,c = LSTMCell(dec_in, h, c; dec_*)      # decoder
      where dec_in == 0 for t==0 and dec_in == h (same tensor!) for t>=1
  out[:, t, :] = h_t @ fc_W.T + fc_b

Key algebraic fusion: for t>=1 the cell input equals the hidden state, so
  gates_t = h_{t-1} @ (dec_Wih + dec_Whh).T + dec_b
and for t==0 (dec_in = 0):
  gates_0 = h_enc @ dec_Whh.T + dec_b

Sharding: pure data-parallel over batch across 8 NeuronCores (64 rows each),
weights replicated.  Per-core layout highlights:
  - matmuls: out = lhsT.T @ rhs with lhsT = transposed hidden state
    (hT, [128 x 64] per 128-hidden chunk), rhs = pre-transposed weights.
  - PE column-tiling: two concurrent M=64 matmuls at tile_position (0,0) and
    (0,64) compute the two hidden-halves of each gate quarter into a "folded"
    [128, 512] PSUM tile (batch duplicated across partition halves), doubling
    PE throughput and letting elementwise ops use all 128 partitions.
  - gate bias rides as a K=1 ones-row matmul that opens each PSUM
    accumulation group; the FC output matmul of the previous step rides in
    each step's stream to fill the recurrence tail bubble.
"""

import os
import sys

import numpy as np

_TRN_REPO = "/opt/trn_rl_repo"
if _TRN_REPO not in sys.path:
    sys.path.insert(0, _TRN_REPO)

B, I, H, O, T = 512, 256, 1024, 128, 100
N_CORES = 8
BQ = B // N_CORES  # 64 batch rows per core
KCH = H // 128     # 8 k-chunks of the hidden dim
G4 = 4 * H         # 4096 gate columns
WALL_N = G4 + O    # gate weights + fc weights, concatenated along columns

_F32 = np.float32
_BF16_VIEW = None  # ml_dtypes.bfloat16, resolved lazily


def _bf16(a):
    import ml_dtypes

    return np.asarray(a, dtype=ml_dtypes.bfloat16)


def build_bass(T_steps=T):
    """Builds the per-core Bass program (same program on all 8 cores)."""
    import concourse.bass as bass
    import concourse.tile as tile
    from concourse import bacc, mybir

    f32 = mybir.dt.float32
    bf16 = mybir.dt.bfloat16
    AF = mybir.ActivationFunctionType

    nc = bacc.Bacc("TRN2", target_bir_lowering=False, debug=False,
                   enable_asserts=False)

    # ---- DRAM I/O ----
    xT_d = nc.dram_tensor("xT", [I, BQ], bf16, kind="ExternalInput").ap()
    encW_d = nc.dram_tensor("encW", [I, G4], bf16, kind="ExternalInput").ap()
    whhT_d = nc.dram_tensor("whhT", [H, G4], bf16, kind="ExternalInput").ap()
    wall_d = nc.dram_tensor("wall", [H, WALL_N], bf16, kind="ExternalInput").ap()
    encb_d = nc.dram_tensor("encb", [1, G4], bf16, kind="ExternalInput").ap()
    decb_d = nc.dram_tensor("decb", [1, WALL_N], bf16, kind="ExternalInput").ap()
    ones_d = nc.dram_tensor("ones", [1, BQ], bf16, kind="ExternalInput").ap()
    ident_d = nc.dram_tensor("ident", [128, 128], bf16, kind="ExternalInput").ap()
    out_d = nc.dram_tensor("out", [BQ, T_steps, O], f32, kind="ExternalOutput").ap()

    QH = 512  # hidden half (columns per folded tile)

    with tile.TileContext(nc) as tc:
        from contextlib import ExitStack

        ctx = ExitStack()
        with ctx:
            # ---- persistent SBUF pools ----
            consts = ctx.enter_context(tc.tile_pool(name="consts", bufs=1))
            wpool = ctx.enter_context(tc.tile_pool(name="wpool", bufs=1))
            wtmp = ctx.enter_context(tc.tile_pool(name="wtmp", bufs=2))
            cpool = ctx.enter_context(tc.tile_pool(name="cpool", bufs=2))
            hpool = ctx.enter_context(tc.tile_pool(name="hpool", bufs=2))
            htpool = ctx.enter_context(tc.tile_pool(name="htpool", bufs=2))
            sgpool = ctx.enter_context(tc.tile_pool(name="sgpool", bufs=2))
            ttpool = ctx.enter_context(tc.tile_pool(name="ttpool", bufs=2))
            fcpool = ctx.enter_context(tc.tile_pool(name="fcpool", bufs=3))
            # PSUM pools (8 banks total: 5 + 1 + 2)
            pg = ctx.enter_context(tc.tile_pool(name="pg", bufs=5, space="PSUM"))
            ptr = ctx.enter_context(tc.tile_pool(name="ptr", bufs=1, space="PSUM"))
            pfc = ctx.enter_context(tc.tile_pool(name="pfc", bufs=2, space="PSUM"))

            # ---- constants / weights into SBUF ----
            ones_sb = consts.tile([1, BQ], bf16, tag="ones")
            nc.sync.dma_start(ones_sb[:], ones_d[:])
            ident_sb = consts.tile([128, 128], bf16, tag="ident")
            nc.sync.dma_start(ident_sb[:], ident_d[:])
            encb_sb = consts.tile([1, G4], bf16, tag="encb")
            nc.sync.dma_start(encb_sb[:], encb_d[:])
            decb_sb = consts.tile([1, WALL_N], bf16, tag="decb")
            nc.sync.dma_start(decb_sb[:], decb_d[:])
            xT_sb = consts.tile([128, 2 * BQ], bf16, tag="xT")
            for k in range(2):
                nc.sync.dma_start(xT_sb[:, k * BQ:(k + 1) * BQ],
                                  xT_d[k * 128:(k + 1) * 128, :])
            encW_sb = consts.tile([128, 2 * G4], bf16, tag="encW")
            for k in range(2):
                nc.sync.dma_start(encW_sb[:, k * G4:(k + 1) * G4],
                                  encW_d[k * 128:(k + 1) * 128, :])

            # main fused weights, resident: [128, KCH * WALL_N] bf16
            wall_sb = wpool.tile([128, KCH * WALL_N], bf16, tag="wall")
            for k in range(KCH):
                nc.sync.dma_start(wall_sb[:, k * WALL_N:(k + 1) * WALL_N],
                                  wall_d[k * 128:(k + 1) * 128, :])

            def wall_rhs(k, col0, ncols):
                return wall_sb[:, k * WALL_N + col0: k * WALL_N + col0 + ncols]

            # ---------------- helpers ----------------
            # quarter order: g first (starts the c-chain early), then i, f, o
            QI, QF, QG, QO = 0, 1, 2, 3

            def bias_mm(p, q, bias_sb):
                """K=1 ones-row matmuls initializing the folded psum tile
                with the per-half gate biases; opens each partition-half's
                accumulation group.  CoreSim's group-check lint mis-addresses
                base-partition-64 matmuls (its data model is correct), so the
                upper-half matmuls skip the lint.
                """
                for half in range(2):
                    col0 = q * H + half * QH
                    nc.tensor.matmul(
                        p[half * 64:(half + 1) * 64, :],
                        ones_sb[:, :],
                        bias_sb[:, col0:col0 + QH],
                        start=True, stop=False,
                        skip_group_check=(half == 1),
                    )

            def gate_quarter(q, lhs_fn, rhs_fn, bias_sb, kch):
                """One gate quarter into a folded [128, 512] psum tile.

                folded rows 0:64   = batch x hidden[q*1024 + 0:512]
                folded rows 64:128 = batch x hidden[q*1024 + 512:1024]
                """
                p = pg.tile([128, QH], f32, tag="pg")
                bias_mm(p, q, bias_sb)
                for k in range(kch):
                    for half in range(2):
                        col0 = q * H + half * QH
                        nc.tensor.matmul(
                            p[half * 64:(half + 1) * 64, :],
                            lhs_fn(k),
                            rhs_fn(k, col0, QH),
                            start=False,
                            stop=(k == kch - 1),
                            skip_group_check=(half == 1),
                        )
                return p

            def act(func, out_t, in_t):
                nc.scalar.activation(out_t[:], in_t[:], func)

            def transposes_and_ht(h_both):
                """h_both [128, 512] bf16 folded -> hT [128, 512] bf16.

                Four full-width [128, 128] PE transposes of the folded h;
                output block j holds hT chunk j in cols [j*128, j*128+64) and
                chunk j+4 in cols [j*128+64, (j+1)*128).
                """
                p = ptr.tile([128, KCH * 64], bf16, tag="ptr")
                for j in range(4):
                    nc.tensor.matmul(
                        p[:, j * 128:(j + 1) * 128],
                        h_both[:, j * 128:(j + 1) * 128],
                        ident_sb[:, :],
                        is_transpose=True, start=True, stop=True,
                    )
                hT = htpool.tile([128, KCH * 64], bf16, tag="hT")
                nc.vector.tensor_copy(hT[:], p[:])
                return hT

            def ht_chunk(hT, k):
                col0 = (k % 4) * 128 + (k // 4) * 64
                return hT[:, col0:col0 + 64]

            def fc_mms(hT_prev):
                p = pfc.tile([64, O], f32, tag="pfc")
                nc.tensor.matmul(p[:, :], ones_sb[:, :],
                                 decb_sb[:, G4:G4 + O], start=True, stop=False)
                for k in range(KCH):
                    nc.tensor.matmul(
                        p[:, :],
                        ht_chunk(hT_prev, k),
                        wall_rhs(k, G4, O),
                        start=False, stop=(k == KCH - 1),
                    )
                return p

            def fc_out(p, t_row):
                sb = fcpool.tile([64, O], f32, tag="fcsb")
                nc.vector.tensor_copy(sb[:], p[:])
                nc.sync.dma_start(out_d[:, t_row, :], sb[:])

            # ---------------- encoder ----------------
            # gates_e = x @ enc_Wih.T + enc_b ; f-gate unused (c_prev = 0)
            def enc_lhs(k):
                return xT_sb[:, k * BQ:(k + 1) * BQ]

            def enc_rhs(k, col0, ncols):
                return encW_sb[:, k * G4 + col0: k * G4 + col0 + ncols]

            p_g = gate_quarter(QG, enc_lhs, enc_rhs, encb_sb, 2)
            s_g = sgpool.tile([128, QH], f32, tag="s_g")
            act(AF.Tanh, s_g, p_g)
            p_i = gate_quarter(QI, enc_lhs, enc_rhs, encb_sb, 2)
            s_i = sgpool.tile([128, QH], f32, tag="s_i")
            act(AF.Sigmoid, s_i, p_i)
            c_cur = cpool.tile([128, QH], f32, tag="c")
            nc.vector.tensor_mul(c_cur[:], s_i[:], s_g[:])
            tc_t = ttpool.tile([128, QH], f32, tag="tc")
            act(AF.Tanh, tc_t, c_cur)
            p_o = gate_quarter(QO, enc_lhs, enc_rhs, encb_sb, 2)
            s_o = sgpool.tile([128, QH], f32, tag="s_o")
            act(AF.Sigmoid, s_o, p_o)
            h_both = hpool.tile([128, QH], bf16, tag="h")
            nc.vector.tensor_mul(h_both[:], s_o[:], tc_t[:])
            hT_cur = transposes_and_ht(h_both)

            # ---------------- decoder steps ----------------
            for t in range(T_steps):
                hT_prev = hT_cur
                c_prev = c_cur

                if t == 0:
                    # gates_0 = h_enc @ dec_Whh.T + dec_b, streaming whhT from
                    # HBM in k-chunk pairs; k-outer so a chunk is used once.
                    pq = [pg.tile([128, QH], f32, tag="pg", name=f"pq{q_}")
                          for q_ in range(4)]
                    for q in range(4):
                        bias_mm(pq[q], q, decb_sb)
                    for pair in range(KCH // 2):
                        wt = wtmp.tile([128, 2 * G4], bf16, tag="wt")
                        for kk in range(2):
                            k = 2 * pair + kk
                            nc.sync.dma_start(
                                wt[:, kk * G4:(kk + 1) * G4],
                                whhT_d[k * 128:(k + 1) * 128, :])
                        for kk in range(2):
                            k = 2 * pair + kk
                            last = k == KCH - 1
                            for q in range(4):
                                for half in range(2):
                                    col0 = q * H + half * QH
                                    nc.tensor.matmul(
                                        pq[q][half * 64:(half + 1) * 64, :],
                                        ht_chunk(hT_prev, k),
                                        wt[:, kk * G4 + col0: kk * G4 + col0 + QH],
                                        start=False, stop=last,
                                        skip_group_check=(half == 1),
                                    )
                    p_g, p_i, p_f, p_o = pq[QG], pq[QI], pq[QF], pq[QO]
                    s_g = sgpool.tile([128, QH], f32, tag="s_g")
                    act(AF.Tanh, s_g, p_g)
                    s_i = sgpool.tile([128, QH], f32, tag="s_i")
                    act(AF.Sigmoid, s_i, p_i)
                    s_f = sgpool.tile([128, QH], f32, tag="s_f")
                    act(AF.Sigmoid, s_f, p_f)
                else:
                    def dec_lhs(k, _h=hT_prev):
                        return ht_chunk(_h, k)

                    p_g = gate_quarter(QG, dec_lhs, wall_rhs, decb_sb, KCH)
                    s_g = sgpool.tile([128, QH], f32, tag="s_g")
                    act(AF.Tanh, s_g, p_g)
                    p_i = gate_quarter(QI, dec_lhs, wall_rhs, decb_sb, KCH)
                    s_i = sgpool.tile([128, QH], f32, tag="s_i")
                    act(AF.Sigmoid, s_i, p_i)
                    p_f = gate_quarter(QF, dec_lhs, wall_rhs, decb_sb, KCH)
                    s_f = sgpool.tile([128, QH], f32, tag="s_f")
                    act(AF.Sigmoid, s_f, p_f)

                t2 = ttpool.tile([128, QH], f32, tag="t2")
                nc.vector.tensor_mul(t2[:], s_i[:], s_g[:])
                t1 = ttpool.tile([128, QH], f32, tag="t1")
                nc.vector.tensor_mul(t1[:], s_f[:], c_prev[:])
                c_cur = cpool.tile([128, QH], f32, tag="c")
                nc.vector.tensor_add(c_cur[:], t1[:], t2[:])
                tc_t = ttpool.tile([128, QH], f32, tag="tc")
                act(AF.Tanh, tc_t, c_cur)

                if t == 0:
                    p_o = pq[QO]
                else:
                    p_o = gate_quarter(QO, dec_lhs, wall_rhs, decb_sb, KCH)
                s_o = sgpool.tile([128, QH], f32, tag="s_o")
                act(AF.Sigmoid, s_o, p_o)
                h_both = hpool.tile([128, QH], bf16, tag="h")
                nc.vector.tensor_mul(h_both[:], s_o[:], tc_t[:])

                # fc for the PREVIOUS step's h rides here (fills the tail
                # bubble while ACT/DVE finish h for this step).
                if t >= 1:
                    p = fc_mms(hT_prev)
                    fc_out(p, t - 1)

                hT_cur = transposes_and_ht(h_both)

            # fc epilogue for the last step's h
            p = fc_mms(hT_cur)
            fc_out(p, T_steps - 1)

    nc.compile()
    return nc


def _prep_inputs(x, enc_Wih, enc_Whh, enc_bih, enc_bhh,
                 dec_Wih, dec_Whh, dec_bih, dec_bhh, fc_W, fc_b):
    """Host-side prep: fuse/transpose/cast; returns per-core in_maps."""
    x = np.asarray(x, _F32)
    wc = np.asarray(dec_Wih, _F32) + np.asarray(dec_Whh, _F32)  # [4H, H]
    wall = np.concatenate([wc.T, np.asarray(fc_W, _F32).T], axis=1)  # [H, 4H+O]
    whhT = np.ascontiguousarray(np.asarray(dec_Whh, _F32).T)  # [H, 4H]
    encW = np.ascontiguousarray(np.asarray(enc_Wih, _F32).T)  # [I, 4H]
    encb = (np.asarray(enc_bih, _F32) + np.asarray(enc_bhh, _F32))[None, :]
    decb = np.concatenate(
        [np.asarray(dec_bih, _F32) + np.asarray(dec_bhh, _F32),
         np.asarray(fc_b, _F32)])[None, :]
    xT = np.ascontiguousarray(x.T)  # [I, B]
    ident = np.eye(128, dtype=_F32)
    ones = np.ones((1, BQ), _F32)

    shared = {
        "encW": _bf16(encW),
        "whhT": _bf16(whhT),
        "wall": _bf16(wall),
        "encb": _bf16(encb),
        "decb": _bf16(decb),
        "ones": _bf16(ones),
        "ident": _bf16(ident),
    }
    in_maps = []
    for c in range(N_CORES):
        m = dict(shared)
        m["xT"] = _bf16(xT[:, c * BQ:(c + 1) * BQ])
        in_maps.append(m)
    return in_maps


_CACHED = {}


def _get_compiled(T_steps=T):
    if T_steps not in _CACHED:
        _CACHED[T_steps] = build_bass(T_steps)
    return _CACHED[T_steps]


def kernel(**inputs):
    from concourse.bass_utils import run_bass_kernel_spmd

    nc = _get_compiled(T)
    in_maps = _prep_inputs(**inputs)
    res = run_bass_kernel_spmd(nc, in_maps, core_ids=list(range(N_CORES)))
    outs = [res.results[c]["out"] for c in range(N_CORES)]
    return np.concatenate(outs, axis=0)  # [B, T, O] fp32


if __name__ == "__main__":
    # quick shape smoke test with random inputs
    rng = np.random.default_rng(0)
    ins = {
        "x": rng.standard_normal((B, I), dtype=_F32),
        "enc_Wih": rng.standard_normal((G4, I), dtype=_F32) * 0.03,
        "enc_Whh": rng.standard_normal((G4, H), dtype=_F32) * 0.03,
        "enc_bih": rng.standard_normal(G4).astype(_F32) * 0.03,
        "enc_bhh": rng.standard_normal(G4).astype(_F32) * 0.03,
        "dec_Wih": rng.standard_normal((G4, H), dtype=_F32) * 0.03,
        "dec_Whh": rng.standard_normal((G4, H), dtype=_F32) * 0.03,
        "dec_bih": rng.standard_normal(G4).astype(_F32) * 0.03,
        "dec_bhh": rng.standard_normal(G4).astype(_F32) * 0.03,
        "fc_W": rng.standard_normal((O, H), dtype=_F32) * 0.03,
        "fc_b": rng.standard_normal(O).astype(_F32) * 0.03,
    }
    out = kernel(**ins)
    print("out", out.shape, out.dtype, float(np.abs(out).mean()))


# revision 27
# speedup vs baseline: 172.1832x; 172.1832x over previous
"""Trainium2 Bass kernel for a seq2seq LSTM (1-step encoder + T-step decoder + FC).

Model (B=512, I=256, H=1024, O=128, T=100):
  h,c   = LSTMCell(x, 0, 0; enc_Wih, enc_Whh, enc_b)          # encoder
  loop t in 0..T-1:  h,c = LSTMCell(dec_in, h, c; dec_*)      # decoder
      where dec_in == 0 for t==0 and dec_in == h (same tensor!) for t>=1
  out[:, t, :] = h_t @ fc_W.T + fc_b

Key algebraic fusion: for t>=1 the cell input equals the hidden state, so
  gates_t = h_{t-1} @ (dec_Wih + dec_Whh).T + dec_b
and for t==0 (dec_in = 0):
  gates_0 = h_enc @ dec_Whh.T + dec_b

Sharding: pure data-parallel over batch across 8 NeuronCores (64 rows each),
weights replicated.  Per-core layout highlights:
  - matmuls: out = lhsT.T @ rhs with lhsT = transposed hidden state
    (hT, [128 x 64] per 128-hidden chunk), rhs = pre-transposed weights.
  - PE column-tiling: two concurrent M=64 matmuls at tile_position (0,0) and
    (0,64) compute the two hidden-halves of each gate quarter into a "folded"
    [128, 512] PSUM tile (batch duplicated across partition halves), doubling
    PE throughput and letting elementwise ops use all 128 partitions.
  - gate bias rides as a K=1 ones-row matmul that opens each PSUM
    accumulation group; the FC output matmul of the previous step rides in
    each step's stream to fill the recurrence tail bubble.
"""

import os
import sys

import numpy as np

_TRN_REPO = "/opt/trn_rl_repo"
if _TRN_REPO not in sys.path:
    sys.path.insert(0, _TRN_REPO)

B, I, H, O, T = 512, 256, 1024, 128, 100
N_CORES = 8
BQ = B // N_CORES  # 64 batch rows per core
KCH = H // 128     # 8 k-chunks of the hidden dim
G4 = 4 * H         # 4096 gate columns
WALL_N = G4 + O    # gate weights + fc weights, concatenated along columns

_F32 = np.float32
_BF16_VIEW = None  # ml_dtypes.bfloat16, resolved lazily


def _bf16(a):
    import ml_dtypes

    return np.asarray(a, dtype=ml_dtypes.bfloat16)


def build_bass(T_steps=T, tiny_out=False):
    """Builds the per-core Bass program (same program on all 8 cores).

    tiny_out=True is a timing-only variant: the DRAM output holds just the
    final step's row so wall-clock measurements aren't dominated by moving
    the [B, T, O] output over the axon tunnel.
    """
    import concourse.bass as bass
    import concourse.tile as tile
    from concourse import bacc, mybir

    f32 = mybir.dt.float32
    bf16 = mybir.dt.bfloat16
    AF = mybir.ActivationFunctionType

    nc = bacc.Bacc("TRN2", target_bir_lowering=False, debug=False,
                   enable_asserts=False)

    # ---- DRAM I/O ----
    xT_d = nc.dram_tensor("xT", [I, BQ], bf16, kind="ExternalInput").ap()
    encW_d = nc.dram_tensor("encW", [I, G4], bf16, kind="ExternalInput").ap()
    whhT_d = nc.dram_tensor("whhT", [H, G4], bf16, kind="ExternalInput").ap()
    wall_d = nc.dram_tensor("wall", [H, WALL_N], bf16, kind="ExternalInput").ap()
    encb_d = nc.dram_tensor("encb", [1, G4], bf16, kind="ExternalInput").ap()
    decb_d = nc.dram_tensor("decb", [1, WALL_N], bf16, kind="ExternalInput").ap()
    ones_d = nc.dram_tensor("ones", [1, BQ], bf16, kind="ExternalInput").ap()
    ident_d = nc.dram_tensor("ident", [128, 128], bf16, kind="ExternalInput").ap()
    out_T = 1 if tiny_out else T_steps
    out_d = nc.dram_tensor("out", [BQ, out_T, O], f32, kind="ExternalOutput").ap()

    QH = 512  # hidden half (columns per folded tile)

    with tile.TileContext(nc) as tc:
        from contextlib import ExitStack

        ctx = ExitStack()
        with ctx:
            # ---- persistent SBUF pools ----
            consts = ctx.enter_context(tc.tile_pool(name="consts", bufs=1))
            wpool = ctx.enter_context(tc.tile_pool(name="wpool", bufs=1))
            wtmp = ctx.enter_context(tc.tile_pool(name="wtmp", bufs=2))
            cpool = ctx.enter_context(tc.tile_pool(name="cpool", bufs=2))
            hpool = ctx.enter_context(tc.tile_pool(name="hpool", bufs=2))
            htpool = ctx.enter_context(tc.tile_pool(name="htpool", bufs=2))
            sgpool = ctx.enter_context(tc.tile_pool(name="sgpool", bufs=2))
            ttpool = ctx.enter_context(tc.tile_pool(name="ttpool", bufs=2))
            fcpool = ctx.enter_context(tc.tile_pool(name="fcpool", bufs=3))
            # PSUM pools (8 banks total: 5 + 1 + 2)
            pg = ctx.enter_context(tc.tile_pool(name="pg", bufs=5, space="PSUM"))
            ptr = ctx.enter_context(tc.tile_pool(name="ptr", bufs=1, space="PSUM"))
            pfc = ctx.enter_context(tc.tile_pool(name="pfc", bufs=2, space="PSUM"))

            # ---- constants / weights into SBUF ----
            ones_sb = consts.tile([1, BQ], bf16, tag="ones")
            nc.sync.dma_start(ones_sb[:], ones_d[:])
            ident_sb = consts.tile([128, 128], bf16, tag="ident")
            nc.sync.dma_start(ident_sb[:], ident_d[:])
            encb_sb = consts.tile([1, G4], bf16, tag="encb")
            nc.sync.dma_start(encb_sb[:], encb_d[:])
            decb_sb = consts.tile([1, WALL_N], bf16, tag="decb")
            nc.sync.dma_start(decb_sb[:], decb_d[:])
            xT_sb = consts.tile([128, 2 * BQ], bf16, tag="xT")
            for k in range(2):
                nc.sync.dma_start(xT_sb[:, k * BQ:(k + 1) * BQ],
                                  xT_d[k * 128:(k + 1) * 128, :])
            encW_sb = consts.tile([128, 2 * G4], bf16, tag="encW")
            for k in range(2):
                nc.sync.dma_start(encW_sb[:, k * G4:(k + 1) * G4],
                                  encW_d[k * 128:(k + 1) * 128, :])

            # main fused weights, resident: [128, KCH * WALL_N] bf16
            wall_sb = wpool.tile([128, KCH * WALL_N], bf16, tag="wall")
            for k in range(KCH):
                nc.sync.dma_start(wall_sb[:, k * WALL_N:(k + 1) * WALL_N],
                                  wall_d[k * 128:(k + 1) * 128, :])

            def wall_rhs(k, col0, ncols):
                return wall_sb[:, k * WALL_N + col0: k * WALL_N + col0 + ncols]

            # ---------------- helpers ----------------
            # quarter order: g first (starts the c-chain early), then i, f, o
            QI, QF, QG, QO = 0, 1, 2, 3

            def bias_mm(p, q, bias_sb):
                """K=1 ones-row matmuls initializing the folded psum tile
                with the per-half gate biases; opens each partition-half's
                accumulation group.  CoreSim's group-check lint mis-addresses
                base-partition-64 matmuls (its data model is correct), so the
                upper-half matmuls skip the lint.
                """
                for half in range(2):
                    col0 = q * H + half * QH
                    nc.tensor.matmul(
                        p[half * 64:(half + 1) * 64, :],
                        ones_sb[:, :],
                        bias_sb[:, col0:col0 + QH],
                        start=True, stop=False,
                        skip_group_check=(half == 1),
                    )

            def gate_quarter(q, lhs_fn, rhs_fn, bias_sb, kch):
                """One gate quarter into a folded [128, 512] psum tile.

                folded rows 0:64   = batch x hidden[q*1024 + 0:512]
                folded rows 64:128 = batch x hidden[q*1024 + 512:1024]
                """
                p = pg.tile([128, QH], f32, tag="pg")
                bias_mm(p, q, bias_sb)
                for k in range(kch):
                    for half in range(2):
                        col0 = q * H + half * QH
                        nc.tensor.matmul(
                            p[half * 64:(half + 1) * 64, :],
                            lhs_fn(k),
                            rhs_fn(k, col0, QH),
                            start=False,
                            stop=(k == kch - 1),
                            skip_group_check=(half == 1),
                        )
                return p

            def act(func, out_t, in_t):
                nc.scalar.activation(out_t[:], in_t[:], func)

            def transposes_and_ht(h_both):
                """h_both [128, 512] bf16 folded -> hT [128, 512] bf16.

                Four full-width [128, 128] PE transposes of the folded h;
                output block j holds hT chunk j in cols [j*128, j*128+64) and
                chunk j+4 in cols [j*128+64, (j+1)*128).
                """
                p = ptr.tile([128, KCH * 64], bf16, tag="ptr")
                for j in range(4):
                    nc.tensor.matmul(
                        p[:, j * 128:(j + 1) * 128],
                        h_both[:, j * 128:(j + 1) * 128],
                        ident_sb[:, :],
                        is_transpose=True, start=True, stop=True,
                    )
                hT = htpool.tile([128, KCH * 64], bf16, tag="hT")
                nc.vector.tensor_copy(hT[:], p[:])
                return hT

            def ht_chunk(hT, k):
                col0 = (k % 4) * 128 + (k // 4) * 64
                return hT[:, col0:col0 + 64]

            def fc_mms(hT_prev):
                p = pfc.tile([64, O], f32, tag="pfc")
                nc.tensor.matmul(p[:, :], ones_sb[:, :],
                                 decb_sb[:, G4:G4 + O], start=True, stop=False)
                for k in range(KCH):
                    nc.tensor.matmul(
                        p[:, :],
                        ht_chunk(hT_prev, k),
                        wall_rhs(k, G4, O),
                        start=False, stop=(k == KCH - 1),
                    )
                return p

            def fc_out(p, t_row):
                sb = fcpool.tile([64, O], f32, tag="fcsb")
                nc.vector.tensor_copy(sb[:], p[:])
                if tiny_out:
                    if t_row == T_steps - 1:
                        nc.sync.dma_start(out_d[:, 0, :], sb[:])
                else:
                    nc.sync.dma_start(out_d[:, t_row, :], sb[:])

            # ---------------- encoder ----------------
            # gates_e = x @ enc_Wih.T + enc_b ; f-gate unused (c_prev = 0)
            def enc_lhs(k):
                return xT_sb[:, k * BQ:(k + 1) * BQ]

            def enc_rhs(k, col0, ncols):
                return encW_sb[:, k * G4 + col0: k * G4 + col0 + ncols]

            p_g = gate_quarter(QG, enc_lhs, enc_rhs, encb_sb, 2)
            s_g = sgpool.tile([128, QH], f32, tag="s_g")
            act(AF.Tanh, s_g, p_g)
            p_i = gate_quarter(QI, enc_lhs, enc_rhs, encb_sb, 2)
            s_i = sgpool.tile([128, QH], f32, tag="s_i")
            act(AF.Sigmoid, s_i, p_i)
            c_cur = cpool.tile([128, QH], f32, tag="c")
            nc.vector.tensor_mul(c_cur[:], s_i[:], s_g[:])
            tc_t = ttpool.tile([128, QH], f32, tag="tc")
            act(AF.Tanh, tc_t, c_cur)
            p_o = gate_quarter(QO, enc_lhs, enc_rhs, encb_sb, 2)
            s_o = sgpool.tile([128, QH], f32, tag="s_o")
            act(AF.Sigmoid, s_o, p_o)
            h_both = hpool.tile([128, QH], bf16, tag="h")
            nc.vector.tensor_mul(h_both[:], s_o[:], tc_t[:])
            hT_cur = transposes_and_ht(h_both)

            # ---------------- decoder steps ----------------
            for t in range(T_steps):
                hT_prev = hT_cur
                c_prev = c_cur

                if t == 0:
                    # gates_0 = h_enc @ dec_Whh.T + dec_b, streaming whhT from
                    # HBM in k-chunk pairs; k-outer so a chunk is used once.
                    pq = [pg.tile([128, QH], f32, tag="pg", name=f"pq{q_}")
                          for q_ in range(4)]
                    for q in range(4):
                        bias_mm(pq[q], q, decb_sb)
                    for pair in range(KCH // 2):
                        wt = wtmp.tile([128, 2 * G4], bf16, tag="wt")
                        for kk in range(2):
                            k = 2 * pair + kk
                            nc.sync.dma_start(
                                wt[:, kk * G4:(kk + 1) * G4],
                                whhT_d[k * 128:(k + 1) * 128, :])
                        for kk in range(2):
                            k = 2 * pair + kk
                            last = k == KCH - 1
                            for q in range(4):
                                for half in range(2):
                                    col0 = q * H + half * QH
                                    nc.tensor.matmul(
                                        pq[q][half * 64:(half + 1) * 64, :],
                                        ht_chunk(hT_prev, k),
                                        wt[:, kk * G4 + col0: kk * G4 + col0 + QH],
                                        start=False, stop=last,
                                        skip_group_check=(half == 1),
                                    )
                    p_g, p_i, p_f, p_o = pq[QG], pq[QI], pq[QF], pq[QO]
                    s_g = sgpool.tile([128, QH], f32, tag="s_g")
                    act(AF.Tanh, s_g, p_g)
                    s_i = sgpool.tile([128, QH], f32, tag="s_i")
                    act(AF.Sigmoid, s_i, p_i)
                    s_f = sgpool.tile([128, QH], f32, tag="s_f")
                    act(AF.Sigmoid, s_f, p_f)
                else:
                    def dec_lhs(k, _h=hT_prev):
                        return ht_chunk(_h, k)

                    p_g = gate_quarter(QG, dec_lhs, wall_rhs, decb_sb, KCH)
                    s_g = sgpool.tile([128, QH], f32, tag="s_g")
                    act(AF.Tanh, s_g, p_g)
                    p_i = gate_quarter(QI, dec_lhs, wall_rhs, decb_sb, KCH)
                    s_i = sgpool.tile([128, QH], f32, tag="s_i")
                    act(AF.Sigmoid, s_i, p_i)
                    p_f = gate_quarter(QF, dec_lhs, wall_rhs, decb_sb, KCH)
                    s_f = sgpool.tile([128, QH], f32, tag="s_f")
                    act(AF.Sigmoid, s_f, p_f)

                t2 = ttpool.tile([128, QH], f32, tag="t2")
                nc.vector.tensor_mul(t2[:], s_i[:], s_g[:])
                t1 = ttpool.tile([128, QH], f32, tag="t1")
                nc.vector.tensor_mul(t1[:], s_f[:], c_prev[:])
                c_cur = cpool.tile([128, QH], f32, tag="c")
                nc.vector.tensor_add(c_cur[:], t1[:], t2[:])
                tc_t = ttpool.tile([128, QH], f32, tag="tc")
                act(AF.Tanh, tc_t, c_cur)

                if t == 0:
                    p_o = pq[QO]
                else:
                    p_o = gate_quarter(QO, dec_lhs, wall_rhs, decb_sb, KCH)
                s_o = sgpool.tile([128, QH], f32, tag="s_o")
                act(AF.Sigmoid, s_o, p_o)
                h_both = hpool.tile([128, QH], bf16, tag="h")
                nc.vector.tensor_mul(h_both[:], s_o[:], tc_t[:])

                # fc for the PREVIOUS step's h rides here (fills the tail
                # bubble while ACT/DVE finish h for this step).
                if t >= 1:
                    p = fc_mms(hT_prev)
                    fc_out(p, t - 1)

                hT_cur = transposes_and_ht(h_both)

            # fc epilogue for the last step's h
            p = fc_mms(hT_cur)
            fc_out(p, T_steps - 1)

    nc.compile()
    return nc


def _prep_inputs(x, enc_Wih, enc_Whh, enc_bih, enc_bhh,
                 dec_Wih, dec_Whh, dec_bih, dec_bhh, fc_W, fc_b):
    """Host-side prep: fuse/transpose/cast; returns per-core in_maps."""
    x = np.asarray(x, _F32)
    wc = np.asarray(dec_Wih, _F32) + np.asarray(dec_Whh, _F32)  # [4H, H]
    wall = np.concatenate([wc.T, np.asarray(fc_W, _F32).T], axis=1)  # [H, 4H+O]
    whhT = np.ascontiguousarray(np.asarray(dec_Whh, _F32).T)  # [H, 4H]
    encW = np.ascontiguousarray(np.asarray(enc_Wih, _F32).T)  # [I, 4H]
    encb = (np.asarray(enc_bih, _F32) + np.asarray(enc_bhh, _F32))[None, :]
    decb = np.concatenate(
        [np.asarray(dec_bih, _F32) + np.asarray(dec_bhh, _F32),
         np.asarray(fc_b, _F32)])[None, :]
    xT = np.ascontiguousarray(x.T)  # [I, B]
    ident = np.eye(128, dtype=_F32)
    ones = np.ones((1, BQ), _F32)

    shared = {
        "encW": _bf16(encW),
        "whhT": _bf16(whhT),
        "wall": _bf16(wall),
        "encb": _bf16(encb),
        "decb": _bf16(decb),
        "ones": _bf16(ones),
        "ident": _bf16(ident),
    }
    in_maps = []
    for c in range(N_CORES):
        m = dict(shared)
        m["xT"] = _bf16(xT[:, c * BQ:(c + 1) * BQ])
        in_maps.append(m)
    return in_maps


_CACHED = {}


def _get_compiled(T_steps=T):
    if T_steps not in _CACHED:
        _CACHED[T_steps] = build_bass(T_steps)
    return _CACHED[T_steps]


def kernel(**inputs):
    from concourse.bass_utils import run_bass_kernel_spmd

    nc = _get_compiled(T)
    in_maps = _prep_inputs(**inputs)
    res = run_bass_kernel_spmd(nc, in_maps, core_ids=list(range(N_CORES)))
    outs = [res.results[c]["out"] for c in range(N_CORES)]
    return np.concatenate(outs, axis=0)  # [B, T, O] fp32


if __name__ == "__main__":
    # quick shape smoke test with random inputs
    rng = np.random.default_rng(0)
    ins = {
        "x": rng.standard_normal((B, I), dtype=_F32),
        "enc_Wih": rng.standard_normal((G4, I), dtype=_F32) * 0.03,
        "enc_Whh": rng.standard_normal((G4, H), dtype=_F32) * 0.03,
        "enc_bih": rng.standard_normal(G4).astype(_F32) * 0.03,
        "enc_bhh": rng.standard_normal(G4).astype(_F32) * 0.03,
        "dec_Wih": rng.standard_normal((G4, H), dtype=_F32) * 0.03,
        "dec_Whh": rng.standard_normal((G4, H), dtype=_F32) * 0.03,
        "dec_bih": rng.standard_normal(G4).astype(_F32) * 0.03,
        "dec_bhh": rng.standard_normal(G4).astype(_F32) * 0.03,
        "fc_W": rng.standard_normal((O, H), dtype=_F32) * 0.03,
        "fc_b": rng.standard_normal(O).astype(_F32) * 0.03,
    }
    out = kernel(**ins)
    print("out", out.shape, out.dtype, float(np.abs(out).mean()))
